# revision 19
# baseline (speedup 1.0000x reference)
"""Causal self-attention (GPT-style block) on 8 Trainium2 NeuronCores.

Sharding: tensor-parallel over heads (16 heads / 8 cores = 2 per core),
c_attn column-parallel from the full input x, attention fully local per
core, c_proj token-parallel after an on-device AllToAll for batches 0-2
and row-parallel (host-summed partials) for batch 3.

Mixed precision (chosen against the 2e-2 gate by numpy simulation of
every quantization spot; measured 1.4e-2 end to end on the real data):
- Score path in fp8e4m3: q/k generation fp8 DoubleRow (w_qk pre-scaled
  x16 on the host for the fp8 subnormal floor, compensated in the exp
  scale), S = K^T.T @ Q^T fp8 DoubleRow with the 64-deep head
  contraction zero-padded in the second k-subtile (the cost model and
  PE charge by output rows only).
- Value path in bf16 (fp8 anywhere on it costs 2.6-3.7e-2): v
  generation emitted token-major (x^T tile stationary) so no PE
  transposes are needed, PV in fat-M orientation (out po[q,65] per key
  tile/head, 2.2x fewer PE rows than the 65-row-out orientation), bf16
  c_proj.
- exp is the only ACT work (~153us busy = the critical path); ep tiles
  are [128, 1024] (key tile x 2 heads) with diagonal tiles clipped at
  the 128-granular diagonal and ranges merged where an extra ACT
  instruction (~185ns) costs more than exp-ing dead columns.
- Normalize via gpsimd InstNormalizeRecip (division + bf16 cast in one
  Pool op); y^T via 4 PE transposes per block.

Scheduling (everything below is about keeping ACT 100% fed, because
exp is the roofline):
- PE work that is not S/PV (stage-1 qkv, c_proj units, row-parallel
  tail) is cut into <=2us closures on a filler queue and drained one
  per key tile, so the in-order PE stream never runs a long burst that
  starves exp of fresh S tiles (a 16-matmul proj burst = 11us ACT gap).
- Stage-1 of token block g+1 is pushed at the start of attention block
  g (double-buffered even/odd slabs), so batch boundaries don't drain
  ACT.
- AllToAll costs 21.5us in the model and the COLLECTIVE_CORES device
  serializes, so exchanges go out every ~2 blocks and proj(k) is
  drained two units later; the last two half-batches skip the
  collective entirely (row-parallel partials summed on the host) so
  the tail doesn't sit on a cold PE behind the last collective.
- The per-block normalize->transpose->y^T chain is deferred a few key
  tiles into the next block so the PE doesn't wait on the Pool/DVE
  chain.
- b_v folds into a host-side output shift (softmax weights sum to 1);
  b_q/b_k ride the stage-1 psum evictions.
"""

import numpy as np
import ml_dtypes

P = 128
B = 4
T = 2048
BT = B * T            # 8192 tokens
C = 1024
KT = C // P           # 8 contraction tiles of 128
KT2 = KT // 2         # 4 DoubleRow pairs
NTB = BT // 512       # 16 token blocks of 512
HD = 64               # head dim
NQ = T // 512         # 4 query blocks per batch
NCORES = 8
TPH = T // NCORES // 2  # 128 tokens per core per half-batch exchange
WS = 16.0             # host prescale on w_q/w_k (fp8 subnormal floor)
SEXP = 0.125 / (WS * WS)
NEXCH = 4             # units 0-3 exchange+proj; units 4-7 row-parallel

E4NP = ml_dtypes.float8_e4m3
BFNP = ml_dtypes.bfloat16

_CACHED = {}


def _exp_ranges(q0):
    # valid score cols per [tile j | 2 heads] psum tile; merged across
    # gaps where the dead rows cost less than an ACT instruction
    if q0 == 0:
        return [(0, 1024)]
    if q0 == 128:
        return [(q0, 1024)]
    return [(q0, 512), (512 + q0, 1024)]


def _build_nc():
    import concourse.mybir as mybir
    import concourse.tile as tile
    from concourse import bacc
    from concourse.masks import make_identity

    f32 = mybir.dt.float32
    bf16 = mybir.dt.bfloat16
    f8 = mybir.dt.float8e4
    EXP = mybir.ActivationFunctionType.Exp
    DR = mybir.MatmulPerfMode.DoubleRow

    nc = bacc.Bacc("TRN2", target_bir_lowering=False, debug=False,
                   num_devices=NCORES)

    xp8 = nc.dram_tensor("xp8", [NTB, P, KT2, 2, 512], f8, kind="ExternalInput")
    xpb = nc.dram_tensor("xpb", [NTB, P, KT, 512], bf16, kind="ExternalInput")
    wq8 = nc.dram_tensor("wq8", [P, KT2, 2, P], f8, kind="ExternalInput")
    wk8 = nc.dram_tensor("wk8", [P, KT2, 2, P], f8, kind="ExternalInput")
    wvb = nc.dram_tensor("wvb", [P, KT, P], bf16, kind="ExternalInput")
    wpb = nc.dram_tensor("wpb", [P, KT, C], bf16, kind="ExternalInput")
    wprb = nc.dram_tensor("wprb", [P, C], bf16, kind="ExternalInput")
    bq = nc.dram_tensor("bq", [P, 1], f32, kind="ExternalInput")
    bk = nc.dram_tensor("bk", [P, 1], f32, kind="ExternalInput")
    # units 0-5 (batches 0-2): fully-reduced rows for my token shard
    yp = nc.dram_tensor("yp", [3, 2, TPH, C], f32, kind="ExternalOutput")
    # batch 3: row-parallel partials over my 128 channels (host sums)
    ypl = nc.dram_tensor("ypl", [T, C], bf16, kind="ExternalOutput")

    with tile.TileContext(nc) as tc:
        with (
            tc.tile_pool(name="const", bufs=1) as const,
            tc.tile_pool(name="slab", bufs=1) as slab,
            tc.tile_pool(name="yt", bufs=2) as yt_pool,
            tc.tile_pool(name="x8", bufs=3) as x8_pool,
            tc.tile_pool(name="xb", bufs=3) as xb_pool,
            tc.tile_pool(name="e", bufs=6) as e_pool,
            tc.tile_pool(name="pb", bufs=2) as posb_pool,
            tc.tile_pool(name="y8", bufs=2) as y8b_pool,
            tc.tile_pool(name="yg", bufs=2) as yg_pool,
            tc.tile_pool(name="ob", bufs=2) as ob_pool,
            tc.tile_pool(name="dram", bufs=1, space="DRAM") as dram_pool,
            tc.tile_pool(name="pss", bufs=2, space="PSUM") as pss_pool,
            tc.tile_pool(name="shp", bufs=2, space="PSUM") as shp_pool,
            tc.tile_pool(name="pop", bufs=1, space="PSUM") as pop_pool,
        ):
            g_in = [dram_pool.tile([NCORES, P, 2, TPH], bf16,
                                   name=f"g_in{k}", tag=f"g_in{k}")
                    for k in range(3)]
            g_out = [dram_pool.tile([NCORES, P, 2, TPH], bf16,
                                    name=f"g_out{k}", tag=f"g_out{k}")
                     for k in range(3)]

            # --- constants / weights ---
            wq8_sb = const.tile([P, KT2, 2, P], f8)
            wk8_sb = const.tile([P, KT2, 2, P], f8)
            wvb_sb = const.tile([P, KT, P], bf16)
            wpb_sb = const.tile([P, KT, C], bf16)
            wprb_sb = const.tile([P, C], bf16)
            bq_sb = const.tile([P, 1], f32)
            bk_sb = const.tile([P, 1], f32)

            # stage-1 slabs, manually double-buffered by batch parity
            qT8 = [slab.tile([P, 2, T], f8, name=f"qT8_{e}", tag=f"qT8_{e}")
                   for e in range(2)]
            kT8 = [slab.tile([P, 2, T], f8, name=f"kT8_{e}", tag=f"kT8_{e}")
                   for e in range(2)]
            vaug = [slab.tile([P, NQ * 4, 2, HD + 1], bf16, name=f"vaug_{e}",
                              tag=f"vaug_{e}") for e in range(2)]

            # startup order matters: the DMA device is serial in the cost
            # model, so the first token block must beat the weight bulk
            xt8_0 = x8_pool.tile([P, KT2, 2, 512], f8, name="xt8_0", tag="xt8")
            xtb_0 = xb_pool.tile([P, KT, 512], bf16, name="xtb_0", tag="xtb")
            nc.sync.dma_start(xt8_0[:], xp8[0])
            nc.sync.dma_start(wq8_sb[:], wq8[:])
            nc.sync.dma_start(bq_sb[:], bq[:])
            nc.sync.dma_start(wk8_sb[:], wk8[:])
            nc.sync.dma_start(bk_sb[:], bk[:])
            nc.sync.dma_start(xtb_0[:], xpb[0])
            nc.sync.dma_start(wvb_sb[:], wvb[:])

            # zero second k-subtile of the even slabs on the (idle) DVE;
            # odd slabs + ones columns can trail on Pool
            nc.vector.memset(qT8[0][:, 1, :], 0.0)
            nc.vector.memset(kT8[0][:, 1, :], 0.0)
            nc.gpsimd.memset(vaug[0][:, :, :, HD:HD + 1], 1.0)
            nc.gpsimd.memset(qT8[1][:, 1, :], 0.0)
            nc.gpsimd.memset(kT8[1][:, 1, :], 0.0)
            nc.gpsimd.memset(vaug[1][:, :, :, HD:HD + 1], 1.0)

            identf = const.tile([P, P], f32)
            make_identity(nc, identf[:])
            identb = const.tile([P, P], bf16)
            nc.vector.tensor_copy(identb[:], identf[:])

            # mask[p, u] = 1.0 if u >= p else 0.0 (upper-right triangle)
            mask_f = const.tile([P, P], f32)
            nc.gpsimd.memset(mask_f[:], 1.0)
            nc.gpsimd.affine_select(
                out=mask_f[:],
                in_=mask_f[:],
                compare_op=mybir.AluOpType.is_ge,
                fill=0.0,
                base=0,
                pattern=[[1, P]],
                channel_multiplier=-1,
            )
            maskb = const.tile([P, P], bf16)
            nc.vector.tensor_copy(maskb[:], mask_f[:])

            wp_loaded = []
            # two filler queues: stage-1 chunks MUST fully drain before the
            # attention block that reads them emits its S tiles (the Tile
            # framework orders by emission, so a read emitted before its
            # writer reads stale SBUF); proj/partial chunks can drain any
            # time after their inputs' emission
            s1q = []
            pjq = []

            def drain_filler():
                if s1q:
                    s1q.pop(0)()
                elif pjq:
                    pjq.pop(0)()

            def drain_all():
                while s1q or pjq:
                    drain_filler()

            xtiles = {}

            def issue_x(g):
                # x DMAs go out ~2 blocks before their chunks can run: a
                # PE instruction whose input hasn't landed head-of-line
                # blocks the whole engine stream
                if g == 0:
                    xtiles[0] = (xt8_0, xtb_0)
                    return
                xt8 = x8_pool.tile([P, KT2, 2, 512], f8, name=f"xt8_{g}",
                                   tag="xt8")
                xtb = xb_pool.tile([P, KT, 512], bf16, name=f"xtb_{g}",
                                   tag="xtb")
                nc.sync.dma_start(xt8[:], xp8[g])
                nc.sync.dma_start(xtb[:], xpb[g])
                xtiles[g] = (xt8, xtb)

            def push_stage1(g):
                b, lb = g // 4, g % 4
                sl = slice(lb * 512, (lb + 1) * 512)
                xt8, xtb = xtiles.pop(g)

                def qk_chunk(w_sb, b_sb, dst):
                    def run():
                        ps = shp_pool.tile([P, 512], f32, tag="shp",
                                           name=f"ps_{g}")
                        for k2 in range(KT2):
                            nc.tensor.matmul(ps[:], w_sb[:, k2], xt8[:, k2],
                                             start=(k2 == 0),
                                             stop=(k2 == KT2 - 1),
                                             perf_mode=DR)
                        nc.vector.tensor_scalar_add(dst[:, 0, sl], ps[:],
                                                    b_sb[:])
                    return run

                def v_chunk(half):
                    # token-major v (x^T tile stationary); single psum
                    # bank per chunk, one start — zero-region covers the
                    # second tt slot
                    def run():
                        vps = shp_pool.tile([P, 2, 2, HD], f32, tag="shp",
                                            name=f"vps_{g}_{half}")
                        for tt2 in range(2):
                            tt = half * 2 + tt2
                            for kt in range(KT):
                                nc.tensor.matmul(
                                    vps[:, tt2],
                                    xtb[:, kt, tt * P:(tt + 1) * P],
                                    wvb_sb[:, kt, :],
                                    start=(tt2 == 0 and kt == 0),
                                    stop=(tt2 == 1 and kt == KT - 1))
                        j0 = lb * 4 + half * 2
                        nc.vector.tensor_copy(
                            vaug[b % 2][:, j0:j0 + 2, :, 0:HD], vps[:])
                    return run

                s1q.append(qk_chunk(wq8_sb, bq_sb, qT8[b % 2]))
                s1q.append(qk_chunk(wk8_sb, bk_sb, kT8[b % 2]))
                s1q.append(v_chunk(0))
                s1q.append(v_chunk(1))

            proj_state = {}

            def start_proj(k):
                # issue the gathered-y load for unit k NOW, on the Pool
                # queue: it waits on collective k without head-of-line
                # blocking the x-stream DMAs on the sync queue
                if not wp_loaded:
                    for kt in range(KT):
                        nc.sync.dma_start(wpb_sb[:, kt], wpb[:, kt])
                    nc.sync.dma_start(wprb_sb[:], wprb[:])
                    wp_loaded.append(true := True)
                yg = yg_pool.tile([P, NCORES, TPH], bf16, tag="yg",
                                  name=f"yg_{k}")
                nc.sync.dma_start(
                    yg[:], g_out[k // 2][:, :, k % 2, :].rearrange(
                        "c p t -> p c t"))
                proj_state[k] = yg

            def push_proj(k):
                # exchange-path c_proj unit k: 4 psum-transient matmul
                # chunks + a final DMA chunk (yg was loaded by start_proj)
                state = {}

                def head():
                    state["ob"] = ob_pool.tile([P, C], f32, tag="ob",
                                               name=f"ob_{k}")

                def mm_chunk(cc):
                    def run():
                        yg, ob = proj_state[k], state["ob"]
                        csl = slice(cc * 256, (cc + 1) * 256)
                        pp = shp_pool.tile([P, 256], f32, tag="shp",
                                           name=f"pp_{k}_{cc}")
                        for ct in range(KT):
                            nc.tensor.matmul(pp[:], yg[:, ct, :],
                                             wpb_sb[:, ct, csl],
                                             start=(ct == 0),
                                             stop=(ct == KT - 1))
                        nc.vector.tensor_copy(ob[:, csl], pp[:])
                    return run

                def finish():
                    nc.sync.dma_start(yp[k // 2, k % 2, :, :], state["ob"])

                pjq.append(head)
                for cc in range(4):
                    pjq.append(mm_chunk(cc))
                pjq.append(finish)

            def push_partial_block(i, yT):
                # row-parallel proj for one 512-token block of batch 3
                # (no collective): my 128 channels x full w_proj rows ->
                # bf16 partials, host sums across cores. Pushed right at
                # each blockend so the tail only carries the last block;
                # each token-tile is two half-chunks so the PE matmul of
                # one half pipelines against the DVE eviction of the other
                r0 = i * 512
                state = {}
                for tt4 in range(4):
                    def runA(tt4=tt4):
                        yTh = yT[:, i // 2, (i % 2) * 4 + tt4, :]
                        state["obl"] = ob_pool.tile([P, C], bf16, tag="obl",
                                                    name=f"obl_{i}_{tt4}")
                        pp0 = shp_pool.tile([P, 512], f32, tag="shp",
                                            name=f"lp0_{i}_{tt4}")
                        nc.tensor.matmul(pp0[:], yTh, wprb_sb[:, 0:512],
                                         start=True, stop=True)
                        nc.vector.tensor_copy(state["obl"][:, 0:512], pp0[:])
                    def runB(tt4=tt4):
                        yTh = yT[:, i // 2, (i % 2) * 4 + tt4, :]
                        pp1 = shp_pool.tile([P, 512], f32, tag="shp",
                                            name=f"lp1_{i}_{tt4}")
                        nc.tensor.matmul(pp1[:], yTh, wprb_sb[:, 512:C],
                                         start=True, stop=True)
                        obl = state["obl"]
                        nc.vector.tensor_copy(obl[:, 512:C], pp1[:])
                        nc.sync.dma_start(
                            ypl[r0 + tt4 * P:r0 + (tt4 + 1) * P, :], obl[:])
                    pjq.append(runA)
                    pjq.append(runB)

            def emit_s(b, i, j):
                # S^T[key, query] for both heads of tile j, fp8 DoubleRow
                d = j - 4 * i
                q0 = max(0, d) * P
                qb_, kb_ = qT8[b % 2], kT8[b % 2]
                psp = pss_pool.tile([P, 1024], f32, tag="pss",
                                    name=f"psp_{b}_{i}_{j}")
                for h in range(2):
                    nc.tensor.matmul(
                        psp[:, 512 * h + q0:512 * h + 512],
                        kb_[HD * h:HD * h + HD, :, j * P:(j + 1) * P],
                        qb_[HD * h:HD * h + HD, :, i * 512 + q0:(i + 1) * 512],
                        start=True, stop=True, perf_mode=DR,
                        tile_position=(HD * h, 0))
                ep = e_pool.tile([P, 1024], bf16, tag="e",
                                 name=f"ep_{b}_{i}_{j}")
                for c0, c1 in _exp_ranges(q0):
                    nc.scalar.activation(ep[:, c0:c1], psp[:, c0:c1], EXP,
                                         scale=SEXP)
                if d >= 0:
                    for h in range(2):
                        msl = slice(512 * h + q0, 512 * h + q0 + P)
                        nc.vector.tensor_mul(ep[:, msl], ep[:, msl], maskb[:])
                return ep

            def emit_pv(b, i, j, ep, po):
                # po[q, 0:64] += E^T(tile j) @ V(tile j); col 64 sums E;
                # single start per psum bank (zero-region covers slots)
                d = j - 4 * i
                for t in range(max(0, d), 4):
                    for h in range(2):
                        nc.tensor.matmul(
                            po[:, h * 4 + t, 0:HD + 1],
                            ep[:, 512 * h + t * P:512 * h + (t + 1) * P],
                            vaug[b % 2][:, j, h, :],
                            start=(j == 0 and t == 0),
                            stop=(j == 4 * i + t))

            def emit_exchange(bb, yTb):
                nc.sync.dma_start(g_in[bb].rearrange("j p h t -> p h j t"),
                                  yTb[:, :, :, :])
                nc.gpsimd.collective_compute(
                    "AllToAll",
                    mybir.AluOpType.bypass,
                    replica_groups=[list(range(NCORES))],
                    ins=[g_in[bb][:]],
                    outs=[g_out[bb][:]],
                )

            pending = []

            def make_blockend(b, i, posb, yT):
                def run():
                    y8b = y8b_pool.tile([P, 4, P], bf16, tag="y8b",
                                        name=f"y8b_{b}_{i}")
                    for t in range(4):
                        for h in range(2):
                            s = h * 4 + t
                            nc.gpsimd.normalize_recip(
                                y8b[:, t, HD * h:HD * h + HD],
                                posb[:, s, 0:HD],
                                posb[:, s, HD:HD + 1])
                    yTp = shp_pool.tile([P, 4, P], bf16, tag="shp",
                                        name=f"yTp_{b}_{i}")
                    for t in range(4):
                        nc.tensor.matmul(yTp[:, t, :], y8b[:, t, :], identb[:],
                                         is_transpose=True,
                                         start=(t == 0), stop=(t == 3))
                    nc.vector.tensor_copy(
                        yT[:, i // 2, (i % 2) * 4:(i % 2) * 4 + 4, :], yTp[:])
                    # batches 0,1: AllToAll + token-parallel proj; yg
                    # loads are issued only once the collective is surely
                    # done (a waiting DMA head-of-line blocks its SEQ).
                    # batches 2,3: row-parallel partials, host-summed.
                    if b == 3:
                        push_partial_block(i, yT)
                    if b == 1 and i == 2:
                        start_proj(0)
                        start_proj(1)
                    if b == 2 and i == 0:
                        push_proj(1)
                    if b == 2 and i == 2:
                        start_proj(2)
                        start_proj(3)
                    if b == 3 and i == 0:
                        push_proj(3)
                    if b == 3 and i == 1:
                        start_proj(4)
                        start_proj(5)
                        push_proj(4)
                    if b == 3 and i == 2:
                        push_proj(5)
                    if i == 3:
                        if b < 3:
                            emit_exchange(b, yT)
                        if b == 1:
                            push_proj(0)
                        if b == 2:
                            push_proj(2)
                        if b == 3:
                            drain_all()
                return run

            issue_x(0)
            issue_x(1)
            push_stage1(0)
            drain_all()          # stage-1 of block 0 runs inline up front
            next_g = 1
            next_dma = 2
            yT = None
            for b in range(B):
                for i in range(NQ):
                    nj = 4 * (i + 1)
                    while s1q:   # this block's q/k/v must be emitted first
                        s1q.pop(0)()
                    if next_g < NTB:
                        push_stage1(next_g)
                        next_g += 1
                    while next_dma < NTB and next_dma <= next_g + 1:
                        issue_x(next_dma)
                        next_dma += 1
                    if i == 0:
                        yT = yt_pool.tile([P, 2, 8, TPH], bf16, tag="yT",
                                          name=f"yT_{b}")
                    po = pop_pool.tile([P, 8, P], f32, tag="po",
                                       name=f"po_{b}_{i}")
                    eps = {}
                    depth = 2
                    for j in range(min(depth, nj)):
                        eps[j] = emit_s(b, i, j)
                    for j in range(nj):
                        if j + depth < nj:
                            eps[j + depth] = emit_s(b, i, j + depth)
                        if j == 3 and pending:
                            pending.pop(0)()
                        drain_filler()
                        emit_pv(b, i, j, eps.pop(j), po)
                    posb = posb_pool.tile([P, 8, HD + 1], f32, tag="posb",
                                          name=f"posb_{b}_{i}")
                    nc.vector.tensor_copy(posb[:], po[:, :, 0:HD + 1])
                    pending.append(make_blockend(b, i, posb, yT))
            while pending:
                pending.pop(0)()
            drain_all()

    nc.compile()
    return nc


def _prep_inputs(x, w_attn, b_attn, w_proj):
    x = np.asarray(x, dtype=np.float32)
    w_attn = np.asarray(w_attn, dtype=np.float32)
    b_attn = np.asarray(b_attn, dtype=np.float32)
    w_proj = np.asarray(w_proj, dtype=np.float32)

    xT = np.ascontiguousarray(x.reshape(BT, C).T)          # [C, BT]
    # xp8[tb, p, k2, s2, c] = xT[k2*256 + s2*128 + p, tb*512 + c]
    xp8 = np.ascontiguousarray(
        xT.reshape(KT2, 2, P, NTB, 512).transpose(3, 2, 0, 1, 4)).astype(E4NP)
    # xpb[tb, p, kt, c] = xT[kt*128 + p, tb*512 + c]
    xpb = np.ascontiguousarray(
        xT.reshape(KT, P, NTB, 512).transpose(2, 1, 0, 3)).astype(BFNP)

    wpb = np.ascontiguousarray(
        w_proj.reshape(KT, P, C).transpose(1, 0, 2)).astype(BFNP)

    in_maps = []
    for c in range(NCORES):
        cols = slice(P * c, P * (c + 1))

        def wslice8(off):
            w = WS * w_attn[:, off + P * c: off + P * (c + 1)]  # [1024, 128]
            return np.ascontiguousarray(
                w.reshape(KT2, 2, P, P).transpose(2, 0, 1, 3)).astype(E4NP)

        wv = w_attn[:, 2 * C + P * c: 2 * C + P * (c + 1)]
        wvb = np.ascontiguousarray(
            wv.reshape(KT, P, P).transpose(1, 0, 2)).astype(BFNP)

        in_maps.append({
            "xp8": xp8,
            "xpb": xpb,
            "wq8": wslice8(0),
            "wk8": wslice8(C),
            "wvb": wvb,
            "wpb": wpb,
            "wprb": np.ascontiguousarray(w_proj[cols, :]).astype(BFNP),
            "bq": (WS * np.ascontiguousarray(b_attn[cols])).reshape(P, 1),
            "bk": (WS * np.ascontiguousarray(
                b_attn[C + P * c: C + P * (c + 1)])).reshape(P, 1),
        })
    return in_maps


def kernel(x, w_attn, b_attn, w_proj, b_proj):
    from concourse.bass_utils import run_bass_kernel_spmd

    if "nc" not in _CACHED:
        _CACHED["nc"] = _build_nc()
    nc = _CACHED["nc"]

    in_maps = _prep_inputs(x, w_attn, b_attn, w_proj)
    res = run_bass_kernel_spmd(nc, in_maps, core_ids=list(range(NCORES)))

    # batches 0-2: core c holds tokens [h*1024 + c*128, +128) of each
    # half h; batch 3 comes back as row-parallel partials (bf16)
    y = np.empty((B, T, C), dtype=np.float32)
    for c in range(NCORES):
        part = res.results[c]["yp"]          # [3, 2, 128, C] f32
        for h in range(2):
            y[:3, h * (T // 2) + c * 128: h * (T // 2) + (c + 1) * 128, :] = \
                part[:, h]
    acc = res.results[0]["ypl"].astype(np.float32)
    for c in range(1, NCORES):
        acc = acc + res.results[c]["ypl"].astype(np.float32)
    y[3] = acc
    # b_v folds into a constant output shift (softmax weights sum to 1)
    bias = np.asarray(b_proj, dtype=np.float32) + \
        np.asarray(b_attn, dtype=np.float32)[2 * C:] @ np.asarray(
            w_proj, dtype=np.float32)
    y += bias
    return y


# revision 20
# speedup vs baseline: 1.0320x; 1.0320x over previous
"""Causal self-attention (GPT-style block) on 8 Trainium2 NeuronCores.

Sharding: tensor-parallel over heads (16 heads / 8 cores = 2 per core),
c_attn column-parallel from the full input x, attention fully local per
core, c_proj token-parallel after an on-device AllToAll for batches 0-2
and row-parallel (host-summed partials) for batch 3.

Mixed precision (chosen against the 2e-2 gate by numpy simulation of
every quantization spot; measured 1.4e-2 end to end on the real data):
- Score path in fp8e4m3: q/k generation fp8 DoubleRow (w_qk pre-scaled
  x16 on the host for the fp8 subnormal floor, compensated in the exp
  scale), S = K^T.T @ Q^T fp8 DoubleRow with the 64-deep head
  contraction zero-padded in the second k-subtile (the cost model and
  PE charge by output rows only).
- Value path in bf16 (fp8 anywhere on it costs 2.6-3.7e-2): v
  generation emitted token-major (x^T tile stationary) so no PE
  transposes are needed, PV in fat-M orientation (out po[q,65] per key
  tile/head, 2.2x fewer PE rows than the 65-row-out orientation), bf16
  c_proj.
- exp is the only ACT work (~153us busy = the critical path); ep tiles
  are [128, 1024] (key tile x 2 heads) with diagonal tiles clipped at
  the 128-granular diagonal and ranges merged where an extra ACT
  instruction (~185ns) costs more than exp-ing dead columns.
- Normalize via gpsimd InstNormalizeRecip (division + bf16 cast in one
  Pool op); y^T via 4 PE transposes per block.

Scheduling (everything below is about keeping ACT 100% fed, because
exp is the roofline):
- PE work that is not S/PV (stage-1 qkv, c_proj units, row-parallel
  tail) is cut into <=2us closures on a filler queue and drained one
  per key tile, so the in-order PE stream never runs a long burst that
  starves exp of fresh S tiles (a 16-matmul proj burst = 11us ACT gap).
- Stage-1 of token block g+1 is pushed at the start of attention block
  g (double-buffered even/odd slabs), so batch boundaries don't drain
  ACT.
- AllToAll costs 21.5us in the model and the COLLECTIVE_CORES device
  serializes, so exchanges go out every ~2 blocks and proj(k) is
  drained two units later; the last two half-batches skip the
  collective entirely (row-parallel partials summed on the host) so
  the tail doesn't sit on a cold PE behind the last collective.
- The per-block normalize->transpose->y^T chain is deferred a few key
  tiles into the next block so the PE doesn't wait on the Pool/DVE
  chain.
- b_v folds into a host-side output shift (softmax weights sum to 1);
  b_q/b_k ride the stage-1 psum evictions.
"""

import numpy as np
import ml_dtypes

P = 128
B = 4
T = 2048
BT = B * T            # 8192 tokens
C = 1024
KT = C // P           # 8 contraction tiles of 128
KT2 = KT // 2         # 4 DoubleRow pairs
NTB = BT // 512       # 16 token blocks of 512
HD = 64               # head dim
NQ = T // 512         # 4 query blocks per batch
NCORES = 8
TPH = T // NCORES // 2  # 128 tokens per core per half-batch exchange
WS = 16.0             # host prescale on w_q/w_k (fp8 subnormal floor)
SEXP = 0.125 / (WS * WS)
NEXCH = 4             # units 0-3 exchange+proj; units 4-7 row-parallel

E4NP = ml_dtypes.float8_e4m3
BFNP = ml_dtypes.bfloat16

_CACHED = {}


def _exp_ranges(q0):
    # valid score cols per [tile j | 2 heads] psum tile; merged across
    # gaps where the dead rows cost less than an ACT instruction
    if q0 == 0:
        return [(0, 1024)]
    if q0 == 128:
        return [(q0, 1024)]
    return [(q0, 512), (512 + q0, 1024)]


def _build_nc():
    import concourse.mybir as mybir
    import concourse.tile as tile
    from concourse import bacc
    from concourse.masks import make_identity

    f32 = mybir.dt.float32
    bf16 = mybir.dt.bfloat16
    f8 = mybir.dt.float8e4
    EXP = mybir.ActivationFunctionType.Exp
    DR = mybir.MatmulPerfMode.DoubleRow

    nc = bacc.Bacc("TRN2", target_bir_lowering=False, debug=False,
                   num_devices=NCORES)

    xp8 = nc.dram_tensor("xp8", [NTB, P, KT2, 2, 512], f8, kind="ExternalInput")
    xpb = nc.dram_tensor("xpb", [NTB, P, KT, 512], bf16, kind="ExternalInput")
    wq8 = nc.dram_tensor("wq8", [P, KT2, 2, P], f8, kind="ExternalInput")
    wk8 = nc.dram_tensor("wk8", [P, KT2, 2, P], f8, kind="ExternalInput")
    wvb = nc.dram_tensor("wvb", [P, KT, P], bf16, kind="ExternalInput")
    wpb = nc.dram_tensor("wpb", [P, KT, C], bf16, kind="ExternalInput")
    wprb = nc.dram_tensor("wprb", [P, C], bf16, kind="ExternalInput")
    bq = nc.dram_tensor("bq", [P, 1], f32, kind="ExternalInput")
    bk = nc.dram_tensor("bk", [P, 1], f32, kind="ExternalInput")
    # units 0-5 (batches 0-2): fully-reduced rows for my token shard
    yp = nc.dram_tensor("yp", [3, 2, TPH, C], f32, kind="ExternalOutput")
    # batch 3: row-parallel partials over my 128 channels (host sums)
    ypl = nc.dram_tensor("ypl", [T, C], bf16, kind="ExternalOutput")

    with tile.TileContext(nc) as tc:
        with (
            tc.tile_pool(name="const", bufs=1) as const,
            tc.tile_pool(name="slab", bufs=1) as slab,
            tc.tile_pool(name="yt", bufs=2) as yt_pool,
            tc.tile_pool(name="x8", bufs=3) as x8_pool,
            tc.tile_pool(name="xb", bufs=3) as xb_pool,
            tc.tile_pool(name="e", bufs=6) as e_pool,
            tc.tile_pool(name="pb", bufs=2) as posb_pool,
            tc.tile_pool(name="y8", bufs=2) as y8b_pool,
            tc.tile_pool(name="yg", bufs=3) as yg_pool,
            tc.tile_pool(name="ob", bufs=4) as ob_pool,
            tc.tile_pool(name="dram", bufs=1, space="DRAM") as dram_pool,
            tc.tile_pool(name="pss", bufs=2, space="PSUM") as pss_pool,
            tc.tile_pool(name="shp", bufs=2, space="PSUM") as shp_pool,
            tc.tile_pool(name="pop", bufs=1, space="PSUM") as pop_pool,
        ):
            g_in = [dram_pool.tile([NCORES, P, 2, TPH], bf16,
                                   name=f"g_in{k}", tag=f"g_in{k}")
                    for k in range(3)]
            g_out = [dram_pool.tile([NCORES, P, 2, TPH], bf16,
                                    name=f"g_out{k}", tag=f"g_out{k}")
                     for k in range(3)]

            # --- constants / weights ---
            wq8_sb = const.tile([P, KT2, 2, P], f8)
            wk8_sb = const.tile([P, KT2, 2, P], f8)
            wvb_sb = const.tile([P, KT, P], bf16)
            wpb_sb = const.tile([P, KT, C], bf16)
            wprb_sb = const.tile([P, C], bf16)
            bq_sb = const.tile([P, 1], f32)
            bk_sb = const.tile([P, 1], f32)

            # stage-1 slabs, manually double-buffered by batch parity
            qT8 = [slab.tile([P, 2, T], f8, name=f"qT8_{e}", tag=f"qT8_{e}")
                   for e in range(2)]
            kT8 = [slab.tile([P, 2, T], f8, name=f"kT8_{e}", tag=f"kT8_{e}")
                   for e in range(2)]
            vaug = [slab.tile([P, NQ * 4, 2, HD + 1], bf16, name=f"vaug_{e}",
                              tag=f"vaug_{e}") for e in range(2)]

            # startup order matters: the DMA device is serial in the cost
            # model, so the first token block must beat the weight bulk
            xt8_0 = x8_pool.tile([P, KT2, 2, 512], f8, name="xt8_0", tag="xt8")
            xtb_0 = xb_pool.tile([P, KT, 512], bf16, name="xtb_0", tag="xtb")
            nc.sync.dma_start(xt8_0[:], xp8[0])
            nc.sync.dma_start(wq8_sb[:], wq8[:])
            nc.sync.dma_start(bq_sb[:], bq[:])
            nc.sync.dma_start(wk8_sb[:], wk8[:])
            nc.sync.dma_start(bk_sb[:], bk[:])
            nc.sync.dma_start(xtb_0[:], xpb[0])
            nc.sync.dma_start(wvb_sb[:], wvb[:])

            # zero second k-subtile of the even slabs on the (idle) DVE;
            # odd slabs + ones columns can trail on Pool
            nc.vector.memset(qT8[0][:, 1, :], 0.0)
            nc.vector.memset(kT8[0][:, 1, :], 0.0)
            nc.gpsimd.memset(vaug[0][:, :, :, HD:HD + 1], 1.0)
            nc.gpsimd.memset(qT8[1][:, 1, :], 0.0)
            nc.gpsimd.memset(kT8[1][:, 1, :], 0.0)
            nc.gpsimd.memset(vaug[1][:, :, :, HD:HD + 1], 1.0)

            identf = const.tile([P, P], f32)
            make_identity(nc, identf[:])
            identb = const.tile([P, P], bf16)
            nc.vector.tensor_copy(identb[:], identf[:])

            # mask[p, u] = 1.0 if u >= p else 0.0 (upper-right triangle)
            mask_f = const.tile([P, P], f32)
            nc.gpsimd.memset(mask_f[:], 1.0)
            nc.gpsimd.affine_select(
                out=mask_f[:],
                in_=mask_f[:],
                compare_op=mybir.AluOpType.is_ge,
                fill=0.0,
                base=0,
                pattern=[[1, P]],
                channel_multiplier=-1,
            )
            maskb = const.tile([P, P], bf16)
            nc.vector.tensor_copy(maskb[:], mask_f[:])

            wp_loaded = []
            # two filler queues: stage-1 chunks MUST fully drain before the
            # attention block that reads them emits its S tiles (the Tile
            # framework orders by emission, so a read emitted before its
            # writer reads stale SBUF); proj/partial chunks can drain any
            # time after their inputs' emission
            s1q = []
            pjq = []

            def drain_filler():
                if s1q:
                    s1q.pop(0)()
                elif pjq:
                    pjq.pop(0)()

            def drain_all():
                while s1q or pjq:
                    drain_filler()

            xtiles = {}

            def issue_x(g):
                # x DMAs go out ~2 blocks before their chunks can run: a
                # PE instruction whose input hasn't landed head-of-line
                # blocks the whole engine stream
                if g == 0:
                    xtiles[0] = (xt8_0, xtb_0)
                    return
                xt8 = x8_pool.tile([P, KT2, 2, 512], f8, name=f"xt8_{g}",
                                   tag="xt8")
                xtb = xb_pool.tile([P, KT, 512], bf16, name=f"xtb_{g}",
                                   tag="xtb")
                nc.sync.dma_start(xt8[:], xp8[g])
                nc.sync.dma_start(xtb[:], xpb[g])
                xtiles[g] = (xt8, xtb)

            def push_stage1(g):
                b, lb = g // 4, g % 4
                sl = slice(lb * 512, (lb + 1) * 512)
                xt8, xtb = xtiles.pop(g)

                def qk_chunk(w_sb, b_sb, dst):
                    def run():
                        ps = shp_pool.tile([P, 512], f32, tag="shp",
                                           name=f"ps_{g}")
                        for k2 in range(KT2):
                            nc.tensor.matmul(ps[:], w_sb[:, k2], xt8[:, k2],
                                             start=(k2 == 0),
                                             stop=(k2 == KT2 - 1),
                                             perf_mode=DR)
                        nc.vector.tensor_scalar_add(dst[:, 0, sl], ps[:],
                                                    b_sb[:])
                    return run

                def v_chunk(half):
                    # token-major v (x^T tile stationary); single psum
                    # bank per chunk, one start — zero-region covers the
                    # second tt slot
                    def run():
                        vps = shp_pool.tile([P, 2, 2, HD], f32, tag="shp",
                                            name=f"vps_{g}_{half}")
                        for tt2 in range(2):
                            tt = half * 2 + tt2
                            for kt in range(KT):
                                nc.tensor.matmul(
                                    vps[:, tt2],
                                    xtb[:, kt, tt * P:(tt + 1) * P],
                                    wvb_sb[:, kt, :],
                                    start=(tt2 == 0 and kt == 0),
                                    stop=(tt2 == 1 and kt == KT - 1))
                        j0 = lb * 4 + half * 2
                        nc.vector.tensor_copy(
                            vaug[b % 2][:, j0:j0 + 2, :, 0:HD], vps[:])
                    return run

                s1q.append(qk_chunk(wq8_sb, bq_sb, qT8[b % 2]))
                s1q.append(qk_chunk(wk8_sb, bk_sb, kT8[b % 2]))
                s1q.append(v_chunk(0))
                s1q.append(v_chunk(1))

            proj_state = {}

            def start_proj(k):
                # issue the gathered-y load for unit k NOW, on the Pool
                # queue: it waits on collective k without head-of-line
                # blocking the x-stream DMAs on the sync queue
                if not wp_loaded:
                    for kt in range(KT):
                        nc.sync.dma_start(wpb_sb[:, kt], wpb[:, kt])
                    nc.sync.dma_start(wprb_sb[:], wprb[:])
                    wp_loaded.append(true := True)
                yg = yg_pool.tile([P, NCORES, TPH], bf16, tag="yg",
                                  name=f"yg_{k}")
                nc.sync.dma_start(
                    yg[:], g_out[k // 2][:, :, k % 2, :].rearrange(
                        "c p t -> p c t"))
                proj_state[k] = yg

            def push_proj(k):
                # exchange-path c_proj unit k: 4 psum-transient matmul
                # chunks + a final DMA chunk (yg was loaded by start_proj)
                state = {}

                def head():
                    state["ob"] = ob_pool.tile([P, C], f32, tag="ob",
                                               name=f"ob_{k}")

                def mm_chunk(cc):
                    def run():
                        yg, ob = proj_state[k], state["ob"]
                        csl = slice(cc * 256, (cc + 1) * 256)
                        pp = shp_pool.tile([P, 256], f32, tag="shp",
                                           name=f"pp_{k}_{cc}")
                        for ct in range(KT):
                            nc.tensor.matmul(pp[:], yg[:, ct, :],
                                             wpb_sb[:, ct, csl],
                                             start=(ct == 0),
                                             stop=(ct == KT - 1))
                        nc.vector.tensor_copy(ob[:, csl], pp[:])
                    return run

                def finish():
                    nc.sync.dma_start(yp[k // 2, k % 2, :, :], state["ob"])

                pjq.append(head)
                for cc in range(4):
                    pjq.append(mm_chunk(cc))
                pjq.append(finish)

            def push_partial_block(i, yT):
                # row-parallel proj for one 512-token block of batch 3
                # (no collective): my 128 channels x full w_proj rows ->
                # bf16 partials, host sums across cores. Pushed right at
                # each blockend so the tail only carries the last block;
                # each token-tile is two half-chunks so the PE matmul of
                # one half pipelines against the DVE eviction of the other
                r0 = i * 512
                state = {}
                for tt4 in range(4):
                    def runA(tt4=tt4):
                        yTh = yT[:, i // 2, (i % 2) * 4 + tt4, :]
                        state["obl"] = ob_pool.tile([P, C], bf16, tag="obl",
                                                    name=f"obl_{i}_{tt4}")
                        pp0 = shp_pool.tile([P, 512], f32, tag="shp",
                                            name=f"lp0_{i}_{tt4}")
                        nc.tensor.matmul(pp0[:], yTh, wprb_sb[:, 0:512],
                                         start=True, stop=True)
                        nc.vector.tensor_copy(state["obl"][:, 0:512], pp0[:])
                    def runB(tt4=tt4):
                        yTh = yT[:, i // 2, (i % 2) * 4 + tt4, :]
                        pp1 = shp_pool.tile([P, 512], f32, tag="shp",
                                            name=f"lp1_{i}_{tt4}")
                        nc.tensor.matmul(pp1[:], yTh, wprb_sb[:, 512:C],
                                         start=True, stop=True)
                        obl = state["obl"]
                        nc.vector.tensor_copy(obl[:, 512:C], pp1[:])
                        nc.sync.dma_start(
                            ypl[r0 + tt4 * P:r0 + (tt4 + 1) * P, :], obl[:])
                    pjq.append(runA)
                    pjq.append(runB)

            def emit_s(b, i, j):
                # S^T[key, query] for both heads of tile j, fp8 DoubleRow
                d = j - 4 * i
                q0 = max(0, d) * P
                qb_, kb_ = qT8[b % 2], kT8[b % 2]
                psp = pss_pool.tile([P, 1024], f32, tag="pss",
                                    name=f"psp_{b}_{i}_{j}")
                for h in range(2):
                    nc.tensor.matmul(
                        psp[:, 512 * h + q0:512 * h + 512],
                        kb_[HD * h:HD * h + HD, :, j * P:(j + 1) * P],
                        qb_[HD * h:HD * h + HD, :, i * 512 + q0:(i + 1) * 512],
                        start=True, stop=True, perf_mode=DR,
                        tile_position=(HD * h, 0))
                ep = e_pool.tile([P, 1024], bf16, tag="e",
                                 name=f"ep_{b}_{i}_{j}")
                for c0, c1 in _exp_ranges(q0):
                    nc.scalar.activation(ep[:, c0:c1], psp[:, c0:c1], EXP,
                                         scale=SEXP)
                if d >= 0:
                    for h in range(2):
                        msl = slice(512 * h + q0, 512 * h + q0 + P)
                        nc.vector.tensor_mul(ep[:, msl], ep[:, msl], maskb[:])
                return ep

            def emit_pv(b, i, j, ep, po):
                # po[q, 0:64] += E^T(tile j) @ V(tile j); col 64 sums E;
                # single start per psum bank (zero-region covers slots)
                d = j - 4 * i
                for t in range(max(0, d), 4):
                    for h in range(2):
                        nc.tensor.matmul(
                            po[:, h * 4 + t, 0:HD + 1],
                            ep[:, 512 * h + t * P:512 * h + (t + 1) * P],
                            vaug[b % 2][:, j, h, :],
                            start=(j == 0 and t == 0),
                            stop=(j == 4 * i + t))

            def emit_exchange(bb, yTb):
                nc.sync.dma_start(g_in[bb].rearrange("j p h t -> p h j t"),
                                  yTb[:, :, :, :])
                nc.gpsimd.collective_compute(
                    "AllToAll",
                    mybir.AluOpType.bypass,
                    replica_groups=[list(range(NCORES))],
                    ins=[g_in[bb][:]],
                    outs=[g_out[bb][:]],
                )

            pending = []

            def make_blockend(b, i, posb, yT):
                def run():
                    y8b = y8b_pool.tile([P, 4, P], bf16, tag="y8b",
                                        name=f"y8b_{b}_{i}")
                    for t in range(4):
                        for h in range(2):
                            s = h * 4 + t
                            nc.gpsimd.normalize_recip(
                                y8b[:, t, HD * h:HD * h + HD],
                                posb[:, s, 0:HD],
                                posb[:, s, HD:HD + 1])
                    yTp = shp_pool.tile([P, 4, P], bf16, tag="shp",
                                        name=f"yTp_{b}_{i}")
                    for t in range(4):
                        nc.tensor.matmul(yTp[:, t, :], y8b[:, t, :], identb[:],
                                         is_transpose=True,
                                         start=(t == 0), stop=(t == 3))
                    nc.vector.tensor_copy(
                        yT[:, i // 2, (i % 2) * 4:(i % 2) * 4 + 4, :], yTp[:])
                    # batches 0,1: AllToAll + token-parallel proj; yg
                    # loads are issued only once the collective is surely
                    # done (a waiting DMA head-of-line blocks its SEQ).
                    # batches 2,3: row-parallel partials, host-summed.
                    if b == 3:
                        push_partial_block(i, yT)
                    if b == 1 and i == 2:
                        start_proj(0)
                        start_proj(1)
                    if b == 2 and i == 0:
                        push_proj(1)
                    if b == 2 and i == 2:
                        start_proj(2)
                        start_proj(3)
                    if b == 3 and i == 0:
                        push_proj(3)
                    if b == 3 and i == 1:
                        start_proj(4)
                        start_proj(5)
                        push_proj(4)
                    if b == 3 and i == 2:
                        push_proj(5)
                    if i == 3:
                        if b < 3:
                            emit_exchange(b, yT)
                        if b == 1:
                            push_proj(0)
                        if b == 2:
                            push_proj(2)
                        if b == 3:
                            drain_all()
                return run

            issue_x(0)
            issue_x(1)
            push_stage1(0)
            drain_all()          # stage-1 of block 0 runs inline up front
            next_g = 1
            next_dma = 2
            yT = None
            for b in range(B):
                for i in range(NQ):
                    nj = 4 * (i + 1)
                    while s1q:   # this block's q/k/v must be emitted first
                        s1q.pop(0)()
                    if next_g < NTB:
                        push_stage1(next_g)
                        next_g += 1
                    while next_dma < NTB and next_dma <= next_g + 1:
                        issue_x(next_dma)
                        next_dma += 1
                    if i == 0:
                        yT = yt_pool.tile([P, 2, 8, TPH], bf16, tag="yT",
                                          name=f"yT_{b}")
                    po = pop_pool.tile([P, 8, P], f32, tag="po",
                                       name=f"po_{b}_{i}")
                    eps = {}
                    depth = 2
                    for j in range(min(depth, nj)):
                        eps[j] = emit_s(b, i, j)
                    for j in range(nj):
                        if j + depth < nj:
                            eps[j + depth] = emit_s(b, i, j + depth)
                        if j == 3 and pending:
                            pending.pop(0)()
                        drain_filler()
                        emit_pv(b, i, j, eps.pop(j), po)
                    posb = posb_pool.tile([P, 8, HD + 1], f32, tag="posb",
                                          name=f"posb_{b}_{i}")
                    nc.vector.tensor_copy(posb[:], po[:, :, 0:HD + 1])
                    pending.append(make_blockend(b, i, posb, yT))
            while pending:
                pending.pop(0)()
            drain_all()

    nc.compile()
    return nc


def _prep_inputs(x, w_attn, b_attn, w_proj):
    x = np.asarray(x, dtype=np.float32)
    w_attn = np.asarray(w_attn, dtype=np.float32)
    b_attn = np.asarray(b_attn, dtype=np.float32)
    w_proj = np.asarray(w_proj, dtype=np.float32)

    xT = np.ascontiguousarray(x.reshape(BT, C).T)          # [C, BT]
    # xp8[tb, p, k2, s2, c] = xT[k2*256 + s2*128 + p, tb*512 + c]
    xp8 = np.ascontiguousarray(
        xT.reshape(KT2, 2, P, NTB, 512).transpose(3, 2, 0, 1, 4)).astype(E4NP)
    # xpb[tb, p, kt, c] = xT[kt*128 + p, tb*512 + c]
    xpb = np.ascontiguousarray(
        xT.reshape(KT, P, NTB, 512).transpose(2, 1, 0, 3)).astype(BFNP)

    wpb = np.ascontiguousarray(
        w_proj.reshape(KT, P, C).transpose(1, 0, 2)).astype(BFNP)

    in_maps = []
    for c in range(NCORES):
        cols = slice(P * c, P * (c + 1))

        def wslice8(off):
            w = WS * w_attn[:, off + P * c: off + P * (c + 1)]  # [1024, 128]
            return np.ascontiguousarray(
                w.reshape(KT2, 2, P, P).transpose(2, 0, 1, 3)).astype(E4NP)

        wv = w_attn[:, 2 * C + P * c: 2 * C + P * (c + 1)]
        wvb = np.ascontiguousarray(
            wv.reshape(KT, P, P).transpose(1, 0, 2)).astype(BFNP)

        in_maps.append({
            "xp8": xp8,
            "xpb": xpb,
            "wq8": wslice8(0),
            "wk8": wslice8(C),
            "wvb": wvb,
            "wpb": wpb,
            "wprb": np.ascontiguousarray(w_proj[cols, :]).astype(BFNP),
            "bq": (WS * np.ascontiguousarray(b_attn[cols])).reshape(P, 1),
            "bk": (WS * np.ascontiguousarray(
                b_attn[C + P * c: C + P * (c + 1)])).reshape(P, 1),
        })
    return in_maps


def kernel(x, w_attn, b_attn, w_proj, b_proj):
    from concourse.bass_utils import run_bass_kernel_spmd

    if "nc" not in _CACHED:
        _CACHED["nc"] = _build_nc()
    nc = _CACHED["nc"]

    in_maps = _prep_inputs(x, w_attn, b_attn, w_proj)
    res = run_bass_kernel_spmd(nc, in_maps, core_ids=list(range(NCORES)))

    # batches 0-2: core c holds tokens [h*1024 + c*128, +128) of each
    # half h; batch 3 comes back as row-parallel partials (bf16)
    y = np.empty((B, T, C), dtype=np.float32)
    for c in range(NCORES):
        part = res.results[c]["yp"]          # [3, 2, 128, C] f32
        for h in range(2):
            y[:3, h * (T // 2) + c * 128: h * (T // 2) + (c + 1) * 128, :] = \
                part[:, h]
    acc = res.results[0]["ypl"].astype(np.float32)
    for c in range(1, NCORES):
        acc = acc + res.results[c]["ypl"].astype(np.float32)
    y[3] = acc
    # b_v folds into a constant output shift (softmax weights sum to 1)
    bias = np.asarray(b_proj, dtype=np.float32) + \
        np.asarray(b_attn, dtype=np.float32)[2 * C:] @ np.asarray(
            w_proj, dtype=np.float32)
    y += bias
    return y


# revision 21
# speedup vs baseline: 1.0787x; 1.0453x over previous
"""Causal self-attention (GPT-style block) on 8 Trainium2 NeuronCores.

Sharding: tensor-parallel over heads (16 heads / 8 cores = 2 per core),
c_attn column-parallel from the full input x, attention fully local per
core, c_proj token-parallel after an on-device AllToAll for batches 0-2
and row-parallel (host-summed partials) for batch 3.

Mixed precision (chosen against the 2e-2 gate by numpy simulation of
every quantization spot; measured 1.4e-2 end to end on the real data):
- Score path in fp8e4m3: q/k generation fp8 DoubleRow (w_qk pre-scaled
  x16 on the host for the fp8 subnormal floor, compensated in the exp
  scale), S = K^T.T @ Q^T fp8 DoubleRow with the 64-deep head
  contraction zero-padded in the second k-subtile (the cost model and
  PE charge by output rows only).
- Value path in bf16 (fp8 anywhere on it costs 2.6-3.7e-2): v
  generation emitted token-major (x^T tile stationary) so no PE
  transposes are needed, PV in fat-M orientation (out po[q,65] per key
  tile/head, 2.2x fewer PE rows than the 65-row-out orientation), bf16
  c_proj.
- exp is the only ACT work (~153us busy = the critical path); ep tiles
  are [128, 1024] (key tile x 2 heads) with diagonal tiles clipped at
  the 128-granular diagonal and ranges merged where an extra ACT
  instruction (~185ns) costs more than exp-ing dead columns.
- Normalize via gpsimd InstNormalizeRecip (division + bf16 cast in one
  Pool op); y^T via 4 PE transposes per block.

Scheduling (everything below is about keeping ACT 100% fed, because
exp is the roofline):
- PE work that is not S/PV (stage-1 qkv, c_proj units, row-parallel
  tail) is cut into <=2us closures on a filler queue and drained one
  per key tile, so the in-order PE stream never runs a long burst that
  starves exp of fresh S tiles (a 16-matmul proj burst = 11us ACT gap).
- Stage-1 of token block g+1 is pushed at the start of attention block
  g (double-buffered even/odd slabs), so batch boundaries don't drain
  ACT.
- AllToAll costs 21.5us in the model and the COLLECTIVE_CORES device
  serializes, so exchanges go out every ~2 blocks and proj(k) is
  drained two units later; the last two half-batches skip the
  collective entirely (row-parallel partials summed on the host) so
  the tail doesn't sit on a cold PE behind the last collective.
- The per-block normalize->transpose->y^T chain is deferred a few key
  tiles into the next block so the PE doesn't wait on the Pool/DVE
  chain.
- b_v folds into a host-side output shift (softmax weights sum to 1);
  b_q/b_k ride the stage-1 psum evictions.
"""

import numpy as np
import ml_dtypes

P = 128
B = 4
T = 2048
BT = B * T            # 8192 tokens
C = 1024
KT = C // P           # 8 contraction tiles of 128
KT2 = KT // 2         # 4 DoubleRow pairs
NTB = BT // 512       # 16 token blocks of 512
HD = 64               # head dim
NQ = T // 512         # 4 query blocks per batch
NCORES = 8
TPH = T // NCORES // 2  # 128 tokens per core per half-batch exchange
WS = 16.0             # host prescale on w_q/w_k (fp8 subnormal floor)
SEXP = 0.125 / (WS * WS)
NEXCH = 4             # units 0-3 exchange+proj; units 4-7 row-parallel

E4NP = ml_dtypes.float8_e4m3
BFNP = ml_dtypes.bfloat16

_CACHED = {}


def _exp_ranges(q0):
    # valid score cols per [tile j | 2 heads] psum tile; merged across
    # gaps where the dead rows cost less than an ACT instruction
    if q0 == 0:
        return [(0, 1024)]
    if q0 == 128:
        return [(q0, 1024)]
    return [(q0, 512), (512 + q0, 1024)]


def _build_nc():
    import concourse.mybir as mybir
    import concourse.tile as tile
    from concourse import bacc
    from concourse.masks import make_identity

    f32 = mybir.dt.float32
    bf16 = mybir.dt.bfloat16
    f8 = mybir.dt.float8e4
    EXP = mybir.ActivationFunctionType.Exp
    DR = mybir.MatmulPerfMode.DoubleRow

    nc = bacc.Bacc("TRN2", target_bir_lowering=False, debug=False,
                   num_devices=NCORES)

    xp8 = nc.dram_tensor("xp8", [NTB, P, KT2, 2, 512], f8, kind="ExternalInput")
    xpb = nc.dram_tensor("xpb", [NTB, P, KT, 512], bf16, kind="ExternalInput")
    wq8 = nc.dram_tensor("wq8", [P, KT2, 2, P], f8, kind="ExternalInput")
    wk8 = nc.dram_tensor("wk8", [P, KT2, 2, P], f8, kind="ExternalInput")
    wvb = nc.dram_tensor("wvb", [P, KT, P], bf16, kind="ExternalInput")
    wpb = nc.dram_tensor("wpb", [P, KT, C], bf16, kind="ExternalInput")
    wprb = nc.dram_tensor("wprb", [P, C], bf16, kind="ExternalInput")
    bq = nc.dram_tensor("bq", [P, 1], f32, kind="ExternalInput")
    bk = nc.dram_tensor("bk", [P, 1], f32, kind="ExternalInput")
    # units 0-5 (batches 0-2): fully-reduced rows for my token shard
    yp = nc.dram_tensor("yp", [3, 2, TPH, C], f32, kind="ExternalOutput")
    # batch 3: row-parallel partials over my 128 channels (host sums)
    ypl = nc.dram_tensor("ypl", [T, C], bf16, kind="ExternalOutput")

    with tile.TileContext(nc) as tc:
        with (
            tc.tile_pool(name="const", bufs=1) as const,
            tc.tile_pool(name="slab", bufs=1) as slab,
            tc.tile_pool(name="yt", bufs=2) as yt_pool,
            tc.tile_pool(name="x8", bufs=3) as x8_pool,
            tc.tile_pool(name="xb", bufs=3) as xb_pool,
            tc.tile_pool(name="e", bufs=6) as e_pool,
            tc.tile_pool(name="pb", bufs=2) as posb_pool,
            tc.tile_pool(name="y8", bufs=2) as y8b_pool,
            tc.tile_pool(name="yg", bufs=3) as yg_pool,
            tc.tile_pool(name="ob", bufs=4) as ob_pool,
            tc.tile_pool(name="dram", bufs=1, space="DRAM") as dram_pool,
            tc.tile_pool(name="pss", bufs=2, space="PSUM") as pss_pool,
            tc.tile_pool(name="shp", bufs=2, space="PSUM") as shp_pool,
            tc.tile_pool(name="pop", bufs=1, space="PSUM") as pop_pool,
        ):
            g_in = [dram_pool.tile([NCORES, P, 2, TPH], bf16,
                                   name=f"g_in{k}", tag=f"g_in{k}")
                    for k in range(3)]
            g_out = [dram_pool.tile([NCORES, P, 2, TPH], bf16,
                                    name=f"g_out{k}", tag=f"g_out{k}")
                     for k in range(3)]

            # --- constants / weights ---
            wq8_sb = const.tile([P, KT2, 2, P], f8)
            wk8_sb = const.tile([P, KT2, 2, P], f8)
            wvb_sb = const.tile([P, KT, P], bf16)
            wpb_sb = const.tile([P, KT, C], bf16)
            wprb_sb = const.tile([P, C], bf16)
            bq_sb = const.tile([P, 1], f32)
            bk_sb = const.tile([P, 1], f32)

            # stage-1 slabs, manually double-buffered by batch parity
            qT8 = [slab.tile([P, 2, T], f8, name=f"qT8_{e}", tag=f"qT8_{e}")
                   for e in range(2)]
            kT8 = [slab.tile([P, 2, T], f8, name=f"kT8_{e}", tag=f"kT8_{e}")
                   for e in range(2)]
            vaug = [slab.tile([P, NQ * 4, 2, HD + 1], bf16, name=f"vaug_{e}",
                              tag=f"vaug_{e}") for e in range(2)]

            # startup order matters: the DMA device is serial in the cost
            # model, so the first token block must beat the weight bulk
            xt8_0 = x8_pool.tile([P, KT2, 2, 512], f8, name="xt8_0", tag="xt8")
            xtb_0 = xb_pool.tile([P, KT, 512], bf16, name="xtb_0", tag="xtb")
            nc.sync.dma_start(xt8_0[:], xp8[0])
            nc.sync.dma_start(wq8_sb[:], wq8[:])
            nc.sync.dma_start(bq_sb[:], bq[:])
            nc.sync.dma_start(wk8_sb[:], wk8[:])
            nc.sync.dma_start(bk_sb[:], bk[:])
            nc.sync.dma_start(xtb_0[:], xpb[0])
            nc.sync.dma_start(wvb_sb[:], wvb[:])

            # zero second k-subtile of the even slabs on the (idle) DVE;
            # odd slabs + ones columns can trail on Pool
            nc.vector.memset(qT8[0][:, 1, :], 0.0)
            nc.vector.memset(kT8[0][:, 1, :], 0.0)
            nc.gpsimd.memset(vaug[0][:, :, :, HD:HD + 1], 1.0)
            nc.gpsimd.memset(qT8[1][:, 1, :], 0.0)
            nc.gpsimd.memset(kT8[1][:, 1, :], 0.0)
            nc.gpsimd.memset(vaug[1][:, :, :, HD:HD + 1], 1.0)

            identf = const.tile([P, P], f32)
            make_identity(nc, identf[:])
            identb = const.tile([P, P], bf16)
            nc.vector.tensor_copy(identb[:], identf[:])

            # mask[p, u] = 1.0 if u >= p else 0.0 (upper-right triangle)
            mask_f = const.tile([P, P], f32)
            nc.gpsimd.memset(mask_f[:], 1.0)
            nc.gpsimd.affine_select(
                out=mask_f[:],
                in_=mask_f[:],
                compare_op=mybir.AluOpType.is_ge,
                fill=0.0,
                base=0,
                pattern=[[1, P]],
                channel_multiplier=-1,
            )
            maskb = const.tile([P, P], bf16)
            nc.vector.tensor_copy(maskb[:], mask_f[:])

            wp_loaded = []
            # two filler queues: stage-1 chunks MUST fully drain before the
            # attention block that reads them emits its S tiles (the Tile
            # framework orders by emission, so a read emitted before its
            # writer reads stale SBUF); proj/partial chunks can drain any
            # time after their inputs' emission
            s1q = []
            pjq = []

            def drain_filler():
                if s1q:
                    s1q.pop(0)()
                elif pjq:
                    pjq.pop(0)()

            def drain_all():
                while s1q or pjq:
                    drain_filler()

            xtiles = {}

            def issue_x(g):
                # x DMAs go out ~2 blocks before their chunks can run: a
                # PE instruction whose input hasn't landed head-of-line
                # blocks the whole engine stream
                if g == 0:
                    xtiles[0] = (xt8_0, xtb_0)
                    return
                xt8 = x8_pool.tile([P, KT2, 2, 512], f8, name=f"xt8_{g}",
                                   tag="xt8")
                xtb = xb_pool.tile([P, KT, 512], bf16, name=f"xtb_{g}",
                                   tag="xtb")
                nc.sync.dma_start(xt8[:], xp8[g])
                nc.sync.dma_start(xtb[:], xpb[g])
                xtiles[g] = (xt8, xtb)

            def push_stage1(g):
                b, lb = g // 4, g % 4
                sl = slice(lb * 512, (lb + 1) * 512)
                xt8, xtb = xtiles.pop(g)

                def qk_chunk(w_sb, b_sb, dst):
                    def run():
                        ps = shp_pool.tile([P, 512], f32, tag="shp",
                                           name=f"ps_{g}")
                        for k2 in range(KT2):
                            nc.tensor.matmul(ps[:], w_sb[:, k2], xt8[:, k2],
                                             start=(k2 == 0),
                                             stop=(k2 == KT2 - 1),
                                             perf_mode=DR)
                        nc.vector.tensor_scalar_add(dst[:, 0, sl], ps[:],
                                                    b_sb[:])
                    return run

                def v_chunk(half):
                    # token-major v (x^T tile stationary); single psum
                    # bank per chunk, one start — zero-region covers the
                    # second tt slot
                    def run():
                        vps = shp_pool.tile([P, 2, 2, HD], f32, tag="shp",
                                            name=f"vps_{g}_{half}")
                        for tt2 in range(2):
                            tt = half * 2 + tt2
                            for kt in range(KT):
                                nc.tensor.matmul(
                                    vps[:, tt2],
                                    xtb[:, kt, tt * P:(tt + 1) * P],
                                    wvb_sb[:, kt, :],
                                    start=(tt2 == 0 and kt == 0),
                                    stop=(tt2 == 1 and kt == KT - 1))
                        j0 = lb * 4 + half * 2
                        nc.vector.tensor_copy(
                            vaug[b % 2][:, j0:j0 + 2, :, 0:HD], vps[:])
                    return run

                s1q.append(qk_chunk(wq8_sb, bq_sb, qT8[b % 2]))
                s1q.append(qk_chunk(wk8_sb, bk_sb, kT8[b % 2]))
                s1q.append(v_chunk(0))
                s1q.append(v_chunk(1))

            proj_state = {}

            def start_proj(k):
                # issue the gathered-y load for unit k NOW, on the Pool
                # queue: it waits on collective k without head-of-line
                # blocking the x-stream DMAs on the sync queue
                if not wp_loaded:
                    for kt in range(KT):
                        nc.sync.dma_start(wpb_sb[:, kt], wpb[:, kt])
                    nc.sync.dma_start(wprb_sb[:], wprb[:])
                    wp_loaded.append(true := True)
                yg = yg_pool.tile([P, NCORES, TPH], bf16, tag="yg",
                                  name=f"yg_{k}")
                nc.sync.dma_start(
                    yg[:], g_out[k // 2][:, :, k % 2, :].rearrange(
                        "c p t -> p c t"))
                proj_state[k] = yg

            def push_proj(k):
                # exchange-path c_proj unit k: 4 psum-transient matmul
                # chunks + a final DMA chunk (yg was loaded by start_proj)
                state = {}

                def head():
                    state["ob"] = ob_pool.tile([P, C], f32, tag="ob",
                                               name=f"ob_{k}")

                def mm_chunk(cc):
                    def run():
                        yg, ob = proj_state[k], state["ob"]
                        csl = slice(cc * 256, (cc + 1) * 256)
                        pp = shp_pool.tile([P, 256], f32, tag="shp",
                                           name=f"pp_{k}_{cc}")
                        for ct in range(KT):
                            nc.tensor.matmul(pp[:], yg[:, ct, :],
                                             wpb_sb[:, ct, csl],
                                             start=(ct == 0),
                                             stop=(ct == KT - 1))
                        nc.vector.tensor_copy(ob[:, csl], pp[:])
                    return run

                def finish():
                    nc.sync.dma_start(yp[k // 2, k % 2, :, :], state["ob"])

                pjq.append(head)
                for cc in range(4):
                    pjq.append(mm_chunk(cc))
                pjq.append(finish)

            def push_partial_block(i, yT):
                # row-parallel proj for one 512-token block of batch 3
                # (no collective): my 128 channels x full w_proj rows ->
                # bf16 partials, host sums across cores. Pushed right at
                # each blockend so the tail only carries the last block;
                # each token-tile is two half-chunks so the PE matmul of
                # one half pipelines against the DVE eviction of the other
                r0 = i * 512
                state = {}
                for tt4 in range(4):
                    def runA(tt4=tt4):
                        yTh = yT[:, i // 2, (i % 2) * 4 + tt4, :]
                        state["obl"] = ob_pool.tile([P, C], bf16, tag="obl",
                                                    name=f"obl_{i}_{tt4}",
                                                    bufs=8)
                        pp0 = shp_pool.tile([P, 512], f32, tag="shp",
                                            name=f"lp0_{i}_{tt4}")
                        nc.tensor.matmul(pp0[:], yTh, wprb_sb[:, 0:512],
                                         start=True, stop=True)
                        nc.vector.tensor_copy(state["obl"][:, 0:512], pp0[:])
                    def runB(tt4=tt4):
                        yTh = yT[:, i // 2, (i % 2) * 4 + tt4, :]
                        pp1 = shp_pool.tile([P, 512], f32, tag="shp",
                                            name=f"lp1_{i}_{tt4}")
                        nc.tensor.matmul(pp1[:], yTh, wprb_sb[:, 512:C],
                                         start=True, stop=True)
                        obl = state["obl"]
                        nc.vector.tensor_copy(obl[:, 512:C], pp1[:])
                        nc.sync.dma_start(
                            ypl[r0 + tt4 * P:r0 + (tt4 + 1) * P, :], obl[:])
                    pjq.append(runA)
                    pjq.append(runB)

            def emit_s(b, i, j):
                # S^T[key, query] for both heads of tile j, fp8 DoubleRow
                d = j - 4 * i
                q0 = max(0, d) * P
                qb_, kb_ = qT8[b % 2], kT8[b % 2]
                psp = pss_pool.tile([P, 1024], f32, tag="pss",
                                    name=f"psp_{b}_{i}_{j}")
                for h in range(2):
                    nc.tensor.matmul(
                        psp[:, 512 * h + q0:512 * h + 512],
                        kb_[HD * h:HD * h + HD, :, j * P:(j + 1) * P],
                        qb_[HD * h:HD * h + HD, :, i * 512 + q0:(i + 1) * 512],
                        start=True, stop=True, perf_mode=DR,
                        tile_position=(HD * h, 0))
                ep = e_pool.tile([P, 1024], bf16, tag="e",
                                 name=f"ep_{b}_{i}_{j}")
                for c0, c1 in _exp_ranges(q0):
                    nc.scalar.activation(ep[:, c0:c1], psp[:, c0:c1], EXP,
                                         scale=SEXP)
                if d >= 0:
                    for h in range(2):
                        msl = slice(512 * h + q0, 512 * h + q0 + P)
                        nc.vector.tensor_mul(ep[:, msl], ep[:, msl], maskb[:])
                return ep

            def emit_pv(b, i, j, ep, po):
                # po[q, 0:64] += E^T(tile j) @ V(tile j); col 64 sums E;
                # single start per psum bank (zero-region covers slots)
                d = j - 4 * i
                for t in range(max(0, d), 4):
                    for h in range(2):
                        nc.tensor.matmul(
                            po[:, h * 4 + t, 0:HD + 1],
                            ep[:, 512 * h + t * P:512 * h + (t + 1) * P],
                            vaug[b % 2][:, j, h, :],
                            start=(j == 0 and t == 0),
                            stop=(j == 4 * i + t))

            def emit_exchange(bb, yTb):
                nc.sync.dma_start(g_in[bb].rearrange("j p h t -> p h j t"),
                                  yTb[:, :, :, :])
                nc.gpsimd.collective_compute(
                    "AllToAll",
                    mybir.AluOpType.bypass,
                    replica_groups=[list(range(NCORES))],
                    ins=[g_in[bb][:]],
                    outs=[g_out[bb][:]],
                )

            pending = []

            def make_blockend(b, i, posb, yT):
                def run():
                    y8b = y8b_pool.tile([P, 4, P], bf16, tag="y8b",
                                        name=f"y8b_{b}_{i}")
                    for t in range(4):
                        for h in range(2):
                            s = h * 4 + t
                            nc.gpsimd.normalize_recip(
                                y8b[:, t, HD * h:HD * h + HD],
                                posb[:, s, 0:HD],
                                posb[:, s, HD:HD + 1])
                    yTp = shp_pool.tile([P, 4, P], bf16, tag="shp",
                                        name=f"yTp_{b}_{i}")
                    for t in range(4):
                        nc.tensor.matmul(yTp[:, t, :], y8b[:, t, :], identb[:],
                                         is_transpose=True,
                                         start=(t == 0), stop=(t == 3))
                    nc.vector.tensor_copy(
                        yT[:, i // 2, (i % 2) * 4:(i % 2) * 4 + 4, :], yTp[:])
                    # batches 0,1: AllToAll + token-parallel proj; yg
                    # loads are issued only once the collective is surely
                    # done (a waiting DMA head-of-line blocks its SEQ).
                    # batches 2,3: row-parallel partials, host-summed.
                    if b == 3:
                        push_partial_block(i, yT)
                    if b == 1 and i == 2:
                        start_proj(0)
                        start_proj(1)
                    if b == 2 and i == 0:
                        push_proj(1)
                    if b == 2 and i == 2:
                        start_proj(2)
                        start_proj(3)
                    if b == 3 and i == 0:
                        push_proj(3)
                    if b == 3 and i == 1:
                        start_proj(4)
                        start_proj(5)
                        push_proj(4)
                    if b == 3 and i == 2:
                        push_proj(5)
                    if i == 3:
                        if b < 3:
                            emit_exchange(b, yT)
                        if b == 1:
                            push_proj(0)
                        if b == 2:
                            push_proj(2)
                        if b == 3:
                            drain_all()
                return run

            issue_x(0)
            issue_x(1)
            push_stage1(0)
            drain_all()          # stage-1 of block 0 runs inline up front
            next_g = 1
            next_dma = 2
            yT = None
            for b in range(B):
                for i in range(NQ):
                    nj = 4 * (i + 1)
                    while s1q:   # this block's q/k/v must be emitted first
                        s1q.pop(0)()
                    if next_g < NTB:
                        push_stage1(next_g)
                        next_g += 1
                    while next_dma < NTB and next_dma <= next_g + 1:
                        issue_x(next_dma)
                        next_dma += 1
                    if i == 0:
                        yT = yt_pool.tile([P, 2, 8, TPH], bf16, tag="yT",
                                          name=f"yT_{b}")
                    po = pop_pool.tile([P, 8, P], f32, tag="po",
                                       name=f"po_{b}_{i}")
                    eps = {}
                    depth = 2
                    for j in range(min(depth, nj)):
                        eps[j] = emit_s(b, i, j)
                    for j in range(nj):
                        if j + depth < nj:
                            eps[j + depth] = emit_s(b, i, j + depth)
                        if j == 3 and pending:
                            pending.pop(0)()
                        drain_filler()
                        emit_pv(b, i, j, eps.pop(j), po)
                    posb = posb_pool.tile([P, 8, HD + 1], f32, tag="posb",
                                          name=f"posb_{b}_{i}")
                    nc.vector.tensor_copy(posb[:], po[:, :, 0:HD + 1])
                    pending.append(make_blockend(b, i, posb, yT))
            while pending:
                pending.pop(0)()
            drain_all()

    nc.compile()
    return nc


def _prep_inputs(x, w_attn, b_attn, w_proj):
    x = np.asarray(x, dtype=np.float32)
    w_attn = np.asarray(w_attn, dtype=np.float32)
    b_attn = np.asarray(b_attn, dtype=np.float32)
    w_proj = np.asarray(w_proj, dtype=np.float32)

    xT = np.ascontiguousarray(x.reshape(BT, C).T)          # [C, BT]
    # xp8[tb, p, k2, s2, c] = xT[k2*256 + s2*128 + p, tb*512 + c]
    xp8 = np.ascontiguousarray(
        xT.reshape(KT2, 2, P, NTB, 512).transpose(3, 2, 0, 1, 4)).astype(E4NP)
    # xpb[tb, p, kt, c] = xT[kt*128 + p, tb*512 + c]
    xpb = np.ascontiguousarray(
        xT.reshape(KT, P, NTB, 512).transpose(2, 1, 0, 3)).astype(BFNP)

    wpb = np.ascontiguousarray(
        w_proj.reshape(KT, P, C).transpose(1, 0, 2)).astype(BFNP)

    in_maps = []
    for c in range(NCORES):
        cols = slice(P * c, P * (c + 1))

        def wslice8(off):
            w = WS * w_attn[:, off + P * c: off + P * (c + 1)]  # [1024, 128]
            return np.ascontiguousarray(
                w.reshape(KT2, 2, P, P).transpose(2, 0, 1, 3)).astype(E4NP)

        wv = w_attn[:, 2 * C + P * c: 2 * C + P * (c + 1)]
        wvb = np.ascontiguousarray(
            wv.reshape(KT, P, P).transpose(1, 0, 2)).astype(BFNP)

        in_maps.append({
            "xp8": xp8,
            "xpb": xpb,
            "wq8": wslice8(0),
            "wk8": wslice8(C),
            "wvb": wvb,
            "wpb": wpb,
            "wprb": np.ascontiguousarray(w_proj[cols, :]).astype(BFNP),
            "bq": (WS * np.ascontiguousarray(b_attn[cols])).reshape(P, 1),
            "bk": (WS * np.ascontiguousarray(
                b_attn[C + P * c: C + P * (c + 1)])).reshape(P, 1),
        })
    return in_maps


def kernel(x, w_attn, b_attn, w_proj, b_proj):
    from concourse.bass_utils import run_bass_kernel_spmd

    if "nc" not in _CACHED:
        _CACHED["nc"] = _build_nc()
    nc = _CACHED["nc"]

    in_maps = _prep_inputs(x, w_attn, b_attn, w_proj)
    res = run_bass_kernel_spmd(nc, in_maps, core_ids=list(range(NCORES)))

    # batches 0-2: core c holds tokens [h*1024 + c*128, +128) of each
    # half h; batch 3 comes back as row-parallel partials (bf16)
    y = np.empty((B, T, C), dtype=np.float32)
    for c in range(NCORES):
        part = res.results[c]["yp"]          # [3, 2, 128, C] f32
        for h in range(2):
            y[:3, h * (T // 2) + c * 128: h * (T // 2) + (c + 1) * 128, :] = \
                part[:, h]
    acc = res.results[0]["ypl"].astype(np.float32)
    for c in range(1, NCORES):
        acc = acc + res.results[c]["ypl"].astype(np.float32)
    y[3] = acc
    # b_v folds into a constant output shift (softmax weights sum to 1)
    bias = np.asarray(b_proj, dtype=np.float32) + \
        np.asarray(b_attn, dtype=np.float32)[2 * C:] @ np.asarray(
            w_proj, dtype=np.float32)
    y += bias
    return y
